# revision 14
# baseline (speedup 1.0000x reference)
"""Trainium2 Bass kernel for nn_MessageFunctionForEvent (GNN message function).

Math: the reference is
    em  = W_e2m @ e_wv[b] + b_e2m          (per-node Linear on edge features)
    nw  = W_n2m @ h_w[b]  + b_n2m          (per-node Linear on node features)
    nv  = W_n2m @ h_v[b]  + b_n2m          (node-level, no n axis)
    msg = Wa @ em + Wb @ nw + (Wc @ nv + b_resize)[:, None]
which collapses (precomposing the tiny 128x128 weights on host) to
    msg[b, :, n] = A @ e_wv[b, :, n] + Bm @ h_w[b, :, n] + c[b]
with A = Wa@W_e2m, Bm = Wb@W_n2m, c[b] = Wa@b_e2m + Wb@b_n2m + Wc@nv[b] + b_resize.

The problem is HBM-bound (per-core traffic >> compute), so the streams are
cast to bf16 on the host: e/h chunks and the two 128x128 weights go over the
wire in bf16, matmuls accumulate in fp32 PSUM, the bias-add writes a bf16
output tile, and the host upcasts the result to fp32. This halves HBM traffic
(61.4MB -> 30.7MB per core) for ~1.3e-3 normed rel error (gate is 2e-2).

Device kernel: a single HWDGE ring tops out ~385 GB/s but both rings
together sustain ~425 GB/s (the SBUF AXI fabric limit), so the two rings
are byte-balanced end-to-end: e chunks on the sync(SP) ring, h chunks on
the scalar(ACT) ring, and each chunk's output halves split across both
rings. The tiny precomposed weights go over the sync ring first (HWDGE,
~0.6us first-byte) so compute unblocks immediately. Two accumulated
128x128 matmuls per 500-col fp32 PSUM bank (all A-passes of a chunk
first — they only need e — then B-passes as h lands), bias-add via
VectorE tensor_scalar_add (PSUM fp32 -> SBUF bf16).
Sharding: batch axis (16 batches -> 2 per core).
"""

import sys

import numpy as np
import ml_dtypes

try:
    from concourse import bacc, mybir
except ImportError:  # bare environment: fall back to the in-container repo
    sys.path.append("/opt/trn_rl_repo")
    from concourse import bacc, mybir
import concourse.tile as tile
from concourse.bass_utils import run_bass_kernel_spmd

B, F, N = 16, 128, 20000
NCORES = 8
BPC = B // NCORES          # batches per core
CH = 4000                  # columns per DMA chunk
NT = 500                   # columns per matmul (fits one 2KB fp32 PSUM bank)
BF16 = np.dtype(ml_dtypes.bfloat16)

_cached_nc = None


def _build():
    global _cached_nc
    if _cached_nc is not None:
        return _cached_nc
    f32 = mybir.dt.float32
    bf16 = mybir.dt.bfloat16
    nc = bacc.Bacc("TRN2", target_bir_lowering=False, debug=False,
                   num_devices=NCORES)
    e_d = nc.dram_tensor("e_wv", (BPC, F, N), bf16, kind="ExternalInput").ap()
    h_d = nc.dram_tensor("h_w", (BPC, F, N), bf16, kind="ExternalInput").ap()
    # packed weights: [AT (128 cols) | BT (128 cols) | cT bitcast to bf16
    # pairs (2*BPC cols)] — one DMA with 520B contiguous lines instead of
    # three tiny transfers whose 256B/8B lines clog the sync ring for ~6us.
    WCOLS = 2 * F + 2 * BPC
    w_d = nc.dram_tensor("wpack", (F, WCOLS), bf16, kind="ExternalInput").ap()
    o_d = nc.dram_tensor("msg", (BPC, F, N), bf16, kind="ExternalOutput").ap()

    # chunk schedule per batch: big streaming chunks, tapered at the very
    # end so the pipeline drains with small PE/DMA quanta instead of one
    # full-size chunk of latency.
    def chunks_for(b):
        if b < BPC - 1:
            return [CH] * (N // CH)
        taper = [1000, 500, 500]
        return [CH] * (N // CH - 1) + [CH - sum(taper)] + taper

    with tile.TileContext(nc) as tc:
        with tc.tile_pool(name="w", bufs=1) as wp, \
             tc.tile_pool(name="eh", bufs=7) as ehp, \
             tc.tile_pool(name="out", bufs=4) as opp, \
             tc.tile_pool(name="ps", bufs=2, space="PSUM") as psp:
            w_t = wp.tile([F, WCOLS], bf16)
            nc.sync.dma_start(w_t[:], w_d[:])
            at_t = w_t[:, 0:F]
            bt_t = w_t[:, F:2 * F]
            c_t = w_t[:, 2 * F:].bitcast(f32)       # [F, BPC] fp32 view

            sched = []
            for b in range(BPC):
                n0 = 0
                for cs in chunks_for(b):
                    sched.append((b, n0, cs))
                    n0 += cs
            tiles = {}

            def load(i):
                b, n0, cs = sched[i]
                e_t = ehp.tile([F, cs], bf16, tag="e")
                h_t = ehp.tile([F, cs], bf16, tag="h")
                nc.sync.dma_start(e_t[:], e_d[b, :, n0:n0 + cs])
                nc.scalar.dma_start(h_t[:], h_d[b, :, n0:n0 + cs])
                tiles[i] = (e_t, h_t)

            def compute_store(i):
                b, n0, cs = sched[i]
                e_t, h_t = tiles.pop(i)
                o_t = opp.tile([F, cs], bf16, tag="o")
                # two halves per chunk, each in a 4-bank PSUM tile so the
                # bias-add is ONE DVE instruction per half (the ~400ns
                # per-instruction DVE overhead made 500-col bias-adds the
                # pipeline bottleneck); halves go to different rings, with
                # the ring order alternating per chunk for balance
                r1, r2 = ((nc.sync, nc.scalar) if i % 2 == 0
                          else (nc.scalar, nc.sync))
                parts = ([(0, cs)] if cs <= NT
                         else [(0, cs // 2), (cs // 2, cs)])
                BANK = 512          # fp32 cols per 2KB PSUM bank
                ps_ts = []
                for lo, hi in parts:
                    p = hi - lo
                    # 4-bank padded tile; matmul segments sit at bank-aligned
                    # 512-col offsets so each 500-col output stays in-bank
                    ps_t = psp.tile([F, 4 * BANK], f32, tag="ps")
                    ps_ts.append(ps_t)
                    nseg = max(p // NT, 1)
                    nt = p // nseg
                    for k in range(nseg):
                        nc.tensor.matmul(ps_t[:, k * BANK:k * BANK + nt],
                                         at_t,
                                         e_t[:, lo + k * nt:lo + (k + 1) * nt],
                                         start=True, stop=False)
                for pi, (lo, hi) in enumerate(parts):
                    p = hi - lo
                    ps_t = ps_ts[pi]
                    nseg = max(p // NT, 1)
                    nt = p // nseg
                    for k in range(nseg):
                        nc.tensor.matmul(ps_t[:, k * BANK:k * BANK + nt],
                                         bt_t,
                                         h_t[:, lo + k * nt:lo + (k + 1) * nt],
                                         start=False, stop=True)
                    # one bias-add instruction per half-chunk, strided view
                    # over the nseg bank-aligned segments; part0 runs on the
                    # DVE, part1 concurrently on the ACT engine — a single
                    # engine (~1.1ns/elem/lane) cannot keep up with the
                    # 425 GB/s stream by itself
                    if nseg == 1:
                        ps_v = ps_t[:, :nt]
                        o_v = o_t[:, lo:hi]
                    else:
                        ps_v = ps_t[:, :nseg * BANK].rearrange(
                            "p (s c) -> p s c", c=BANK)[:, :, :nt]
                        o_v = o_t[:, lo:hi].rearrange(
                            "p (s c) -> p s c", c=nt)
                    if pi == 0:
                        nc.vector.tensor_scalar_add(o_v, ps_v,
                                                    c_t[:, b:b + 1])
                    else:
                        nc.scalar.activation(
                            o_v, ps_v, mybir.ActivationFunctionType.Identity,
                            bias=c_t[:, b:b + 1])
                    ring = (r1, r2)[pi % 2]
                    ring.dma_start(o_d[b, :, n0 + lo:n0 + hi],
                                   o_t[:, lo:hi])

            # software-pipelined trigger order: keep LOOK chunks of input
            # loads queued on each ring ahead of the compute/store triggers,
            # so an output trigger waiting on the DVE never starves the ring.
            LOOK = 4
            for i in range(len(sched)):
                load(i)
                if i >= LOOK:
                    compute_store(i - LOOK)
            for i in range(len(sched) - LOOK, len(sched)):
                compute_store(i)
    nc.finalize()
    _cached_nc = nc
    return nc


def _prepare_in_maps(h_w, h_v, e_wv, W_e2m, b_e2m, W_n2m, b_n2m,
                     W_resize, b_resize):
    f64 = np.float64
    M = F
    Wa = W_resize[:, :M].astype(f64)
    Wb = W_resize[:, M:2 * M].astype(f64)
    Wc = W_resize[:, 2 * M:].astype(f64)
    A = Wa @ W_e2m.astype(f64)
    Bm = Wb @ W_n2m.astype(f64)
    nv = h_v.astype(f64) @ W_n2m.astype(f64).T + b_n2m.astype(f64)
    c = (Wa @ b_e2m.astype(f64) + Wb @ b_n2m.astype(f64)
         + nv @ Wc.T + b_resize.astype(f64))          # [B, M]
    AT = np.ascontiguousarray(A.T).astype(BF16)
    BT = np.ascontiguousarray(Bm.T).astype(BF16)
    cT = np.ascontiguousarray(c.T).astype(np.float32)  # [M, B]

    e_bf = e_wv.astype(BF16)
    h_bf = h_w.astype(BF16)
    in_maps = []
    for cid in range(NCORES):
        bs = slice(cid * BPC, (cid + 1) * BPC)
        # pack [AT | BT | cT-bitcast] into one bf16 tensor (see _build)
        c_u16 = np.ascontiguousarray(cT[:, bs]).view(np.uint16)  # [F, 2*BPC]
        wpack = np.concatenate(
            [AT.view(np.uint16), BT.view(np.uint16), c_u16],
            axis=1).view(BF16)
        in_maps.append({
            "e_wv": np.ascontiguousarray(e_bf[bs]),
            "h_w": np.ascontiguousarray(h_bf[bs]),
            "wpack": np.ascontiguousarray(wpack),
        })
    return in_maps


def kernel(**inputs):
    args = {k: np.asarray(inputs[k], dtype=np.float32)
            for k in ("h_w", "h_v", "e_wv", "W_e2m", "b_e2m", "W_n2m",
                      "b_n2m", "W_resize", "b_resize")}
    in_maps = _prepare_in_maps(**args)
    nc = _build()
    res = run_bass_kernel_spmd(nc, in_maps, core_ids=list(range(NCORES)))
    return np.concatenate(
        [r["msg"].astype(np.float32) for r in res.results], axis=0)
